# revision 1
# baseline (speedup 1.0000x reference)
"""Trainium2 Bass kernel for nn_NodeInfoPropagate (GNN message passing).

Strategy (8 NeuronCores, node-parallel):
  - Shard the 20000 nodes across 8 cores (2500/core, padded to 2560 = 5 tiles
    of 512).  Weights replicated.
  - Activations live on-chip in "transposed" layout [feature-on-partition,
    node-on-free], so every matmul chains with zero transposes
    (out = W.T.T @ xT).  fp32 GRU path uses float32r matmuls (full speed for
    moving dim >= 256).
  - Per layer, the full x table [20000+1, 256] is materialized in bf16 in each
    core's HBM via AllGather; parent + neighbor rows are fetched with
    dma_gather(transpose=True, single_packet=False), which lands gathered rows directly in the
    transposed layout.  Invalid (-1) neighbors are pointed at an all-zero
    table row, so no masking is needed; the mean uses a host-precomputed
    1/count broadcast.
  - gather commutes with the linear maps:  p[parent] = x[parent] @ Wp.T and
    sum_k nbr[idx_k] = (sum_k x[idx_k]) @ Wn.T, so only ONE table (x) is ever
    gathered, and the parent/neighbor-mean matmuls accumulate into the same
    PSUM bank (summary = x[par] @ Wp.T + mean @ Wn.T + b_p + b_n).
"""

import sys

sys.path.insert(0, "/opt/trn_rl_repo")

import numpy as np
import ml_dtypes

import concourse.bass as bass
import concourse.bacc as bacc
import concourse.tile as tile
import concourse.mybir as mybir
from concourse import bass_utils

N = 20000
K = 16
H = 256
DIN = 256
NCORES = 8
NC_REAL = N // NCORES          # 2500 real nodes per core
NT = 512                       # node tile (matmul free dim / PSUM bank)
T = 5                          # tiles per core
NCP = NT * T                   # 2560 padded nodes per core
ZROW = N                       # all-zero table row for invalid neighbors
NTAB = N + 128                 # table rows (pad keeps alignment comfy)
NHALF = (NT // 2) * K          # 4096 neighbor idxs per half-tile

F32 = mybir.dt.float32
F32R = mybir.dt.float32r
BF16 = mybir.dt.bfloat16
I16 = mybir.dt.int16
BF = ml_dtypes.bfloat16

_CACHE = {}


def _build(depth: int):
    nc = bacc.Bacc("TRN2", target_bir_lowering=False, debug=False,
                   num_devices=NCORES)

    featT = nc.dram_tensor("featT", [128, 2, NCP], F32, kind="ExternalInput")
    invcnt = nc.dram_tensor("invcnt", [128, NCP], F32, kind="ExternalInput")
    nbr_idx = nc.dram_tensor("nbr_idx", [128, T, NT], I16, kind="ExternalInput")
    par_idx = nc.dram_tensor("par_idx", [128, T, NT // 16], I16, kind="ExternalInput")
    w_in = nc.dram_tensor("w_in", [128, 2, H], F32, kind="ExternalInput")
    w_ih = nc.dram_tensor("w_ih", [128, 2, 3 * H], F32, kind="ExternalInput")
    w_hh = nc.dram_tensor("w_hh", [128, 2, 3 * H], F32, kind="ExternalInput")
    w_p = nc.dram_tensor("w_p", [128, 2, H], BF16, kind="ExternalInput")
    w_n = nc.dram_tensor("w_n", [128, 2, H], BF16, kind="ExternalInput")
    # bias columns: 0-1 b_in, 2-3 b_p+b_n, 4-5 b_r, 6-7 b_z, 8-9 b_ih_n,
    # 10-11 b_hh_n  (per 128-feature chunk)
    biases = nc.dram_tensor("biases", [128, 12], F32, kind="ExternalInput")
    ident_b = nc.dram_tensor("ident_b", [128, 128], BF16, kind="ExternalInput")
    ident_f = nc.dram_tensor("ident_f", [128, 128], F32, kind="ExternalInput")
    y = nc.dram_tensor("y", [NCP, H], F32, kind="ExternalOutput")

    SIG = mybir.ActivationFunctionType.Sigmoid
    TANH = mybir.ActivationFunctionType.Tanh
    ADD = mybir.AluOpType.add
    MULT = mybir.AluOpType.mult

    with tile.TileContext(nc) as tc:
        with (
            tc.tile_pool(name="const", bufs=1) as constp,
            tc.tile_pool(name="state", bufs=1) as statep,
            tc.tile_pool(name="dram", bufs=1, space="DRAM") as dramp,
            tc.tile_pool(name="gath", bufs=2) as gathp,
            tc.tile_pool(name="work", bufs=2) as workp,
            tc.tile_pool(name="tmp", bufs=3) as tmpp,
            tc.tile_pool(name="ps", bufs=2, space="PSUM") as psp,
            tc.tile_pool(name="psg", bufs=6, space="PSUM") as psgp,
        ):
            # ---- resident constants -------------------------------------
            win_sb = constp.tile([128, 2, H], F32R, name="win_sb")
            nc.sync.dma_start(win_sb[:], w_in.ap().bitcast(F32R))
            wih_sb = constp.tile([128, 2, 3 * H], F32R, name="wih_sb")
            nc.sync.dma_start(wih_sb[:], w_ih.ap().bitcast(F32R))
            whh_sb = constp.tile([128, 2, 3 * H], F32R, name="whh_sb")
            nc.sync.dma_start(whh_sb[:], w_hh.ap().bitcast(F32R))
            wp_sb = constp.tile([128, 2, H], BF16, name="wp_sb")
            nc.sync.dma_start(wp_sb[:], w_p.ap())
            wn_sb = constp.tile([128, 2, H], BF16, name="wn_sb")
            nc.sync.dma_start(wn_sb[:], w_n.ap())
            bias_sb = constp.tile([128, 12], F32, name="bias_sb")
            nc.sync.dma_start(bias_sb[:], biases.ap())
            idb_sb = constp.tile([128, 128], BF16, name="idb_sb")
            nc.sync.dma_start(idb_sb[:], ident_b.ap())
            idf_sb = constp.tile([128, 128], F32, name="idf_sb")
            nc.sync.dma_start(idf_sb[:], ident_f.ap())
            feat_sb = constp.tile([128, 2, NCP], F32R, name="feat_sb")
            nc.sync.dma_start(feat_sb[:], featT.ap().bitcast(F32R))
            inv_sb = constp.tile([128, NCP], F32, name="inv_sb")
            nc.sync.dma_start(inv_sb[:], invcnt.ap())
            nbr_sb = constp.tile([128, T, NT], I16, name="nbr_sb")
            nc.sync.dma_start(nbr_sb[:], nbr_idx.ap())
            par_sb = constp.tile([128, T, NT // 16], I16, name="par_sb")
            nc.sync.dma_start(par_sb[:], par_idx.ap())

            xF = [statep.tile([128, 2, NCP], F32R, name=f"xF{i}") for i in range(2)]

            xloc = dramp.tile([NCP, H], BF16, name="xloc")
            xtab = dramp.tile([NTAB, H], BF16, name="xtab")

            # zero row for invalid-neighbor gathers
            zero_sb = constp.tile([128, H], BF16, name="zero_sb")
            nc.vector.memset(zero_sb[:], 0.0)
            nc.sync.dma_start(xtab[ZROW:ZROW + 1, :], zero_sb[0:1, :])

            def mm_f32r(out_ps, lhsT, rhs, start, stop):
                nc.tensor.matmul(out_ps, lhsT, rhs, start=start, stop=stop)

            def write_table_tile(xf, t):
                """cast tile t of xf to bf16, transpose to row-major, DMA to
                xloc rows."""
                ts = slice(t * NT, (t + 1) * NT)
                xb = workp.tile([128, 2, NT], BF16, tag="xb", name="xb")
                nc.vector.tensor_copy(xb[:], xf[:, :, ts].bitcast(F32))
                for b in range(NT // 128):
                    rm = workp.tile([128, 2, 128], BF16, tag="rm", name="rm")
                    for c in range(2):
                        pst = psp.tile([128, 128], BF16, tag="sum", name="pst")
                        nc.tensor.transpose(pst[:], xb[:, c, b * 128:(b + 1) * 128],
                                            idb_sb[:])
                        nc.vector.tensor_copy(rm[:, c, :], pst[:])
                    r0 = t * NT + b * 128
                    nc.sync.dma_start(xloc[r0:r0 + 128, :], rm[:])

            def write_output_tile(xf, t):
                ts0 = t * NT
                for b in range(NT // 128):
                    rmf = workp.tile([128, 2, 128], F32, tag="rmf", name="rmf")
                    for c in range(2):
                        pst = psp.tile([128, 128], F32, tag="sum", name="pstf")
                        nc.tensor.transpose(
                            pst[:], xf[:, c, ts0 + b * 128:ts0 + (b + 1) * 128].bitcast(F32),
                            idf_sb[:])
                        nc.vector.tensor_copy(rmf[:, c, :], pst[:])
                    r0 = ts0 + b * 128
                    nc.sync.dma_start(y[r0:r0 + 128, :], rmf[:])

            def allgather():
                nc.gpsimd.collective_compute(
                    "AllGather", mybir.AluOpType.bypass,
                    replica_groups=[list(range(NCORES))],
                    ins=[xloc[0:NC_REAL, :].opt()],
                    outs=[xtab[0:N, :].opt()],
                )

            # ---- layer 0: x0 = W_in @ feat + b_in ------------------------
            for t in range(T):
                ts = slice(t * NT, (t + 1) * NT)
                for oc in range(2):
                    ps = psp.tile([128, NT], F32, tag="sum", name="ps0")
                    for dc in range(2):
                        mm_f32r(ps[:], win_sb[:, dc, oc * 128:(oc + 1) * 128],
                                feat_sb[:, dc, ts], start=(dc == 0), stop=(dc == 1))
                    nc.vector.tensor_scalar_add(xF[0][:, oc, ts], ps[:],
                                                bias_sb[:, oc:oc + 1])
                if depth == 0:
                    write_output_tile(xF[0], t)
                else:
                    write_table_tile(xF[0], t)
            if depth > 0:
                allgather()

            # ---- GRU layers ---------------------------------------------
            cur = 0
            for layer in range(depth):
                last = layer == depth - 1
                xf_in, xf_out = xF[cur], xF[1 - cur]
                for t in range(T):
                    ts = slice(t * NT, (t + 1) * NT)
                    # parent rows
                    pgat = gathp.tile([128, 2, NT], BF16, tag="pgat", name="pgat")
                    nc.gpsimd.dma_gather(pgat[:], xtab[:], par_sb[:, t, :],
                                         NT, NT, H, transpose=True, single_packet=False)
                    # neighbor rows, two half-tiles; sum groups of K=16
                    nsum = workp.tile([128, 2, NT], F32, tag="nsum", name="nsum")
                    for hf in range(2):
                        hs = slice(hf * (NT // 2), (hf + 1) * (NT // 2))
                        ngat = gathp.tile([128, 2, NHALF], BF16, tag="ngat",
                                          name="ngat")
                        nc.gpsimd.dma_gather(
                            ngat[:], xtab[:],
                            nbr_sb[:, t, hf * (NT // 2):(hf + 1) * (NT // 2)],
                            NHALF, NHALF, H, transpose=True, single_packet=False)
                        for c in range(2):
                            nc.vector.tensor_reduce(
                                nsum[:, c, hs],
                                ngat[:, c, :].rearrange("p (n k) -> p n k", k=K),
                                axis=mybir.AxisListType.X, op=ADD)
                    nmean = workp.tile([128, 2, NT], BF16, tag="nmean", name="nmean")
                    for c in range(2):
                        nc.vector.tensor_mul(nmean[:, c, :], nsum[:, c, :],
                                             inv_sb[:, ts])
                    # summary = pgat @ Wp.T + nmean @ Wn.T + (b_p + b_n)
                    sT = workp.tile([128, 2, NT], F32R, tag="sT", name="sT")
                    for oc in range(2):
                        ps = psp.tile([128, NT], F32, tag="sum", name="psS")
                        for hc in range(2):
                            nc.tensor.matmul(ps[:],
                                             wp_sb[:, hc, oc * 128:(oc + 1) * 128],
                                             pgat[:, hc, :],
                                             start=(hc == 0), stop=False)
                        for hc in range(2):
                            nc.tensor.matmul(ps[:],
                                             wn_sb[:, hc, oc * 128:(oc + 1) * 128],
                                             nmean[:, hc, :],
                                             start=False, stop=(hc == 1))
                        nc.vector.tensor_scalar_add(sT[:, oc, :], ps[:],
                                                    bias_sb[:, 2 + oc:3 + oc])
                    # GRU gates, per output chunk
                    for oc in range(2):
                        rp = psgp.tile([128, NT], F32, tag="gate", name="rp")
                        zp = psgp.tile([128, NT], F32, tag="gate", name="zp")
                        ip = psgp.tile([128, NT], F32, tag="gate", name="ip")
                        hp = psgp.tile([128, NT], F32, tag="gate", name="hp")
                        for gate, pst in ((0, rp), (1, zp)):
                            o0 = gate * H + oc * 128
                            for hc in range(2):
                                mm_f32r(pst[:], wih_sb[:, hc, o0:o0 + 128],
                                        xf_in[:, hc, ts],
                                        start=(hc == 0), stop=False)
                            for hc in range(2):
                                mm_f32r(pst[:], whh_sb[:, hc, o0:o0 + 128],
                                        sT[:, hc, :],
                                        start=False, stop=(hc == 1))
                        o0 = 2 * H + oc * 128
                        for hc in range(2):
                            mm_f32r(ip[:], wih_sb[:, hc, o0:o0 + 128],
                                    xf_in[:, hc, ts],
                                    start=(hc == 0), stop=(hc == 1))
                        for hc in range(2):
                            mm_f32r(hp[:], whh_sb[:, hc, o0:o0 + 128],
                                    sT[:, hc, :],
                                    start=(hc == 0), stop=(hc == 1))
                        r = tmpp.tile([128, NT], F32, tag="r", name="r")
                        nc.scalar.activation(r[:], rp[:], SIG,
                                             bias=bias_sb[:, 4 + oc:5 + oc])
                        z = tmpp.tile([128, NT], F32, tag="z", name="z")
                        nc.scalar.activation(z[:], zp[:], SIG,
                                             bias=bias_sb[:, 6 + oc:7 + oc])
                        # n = tanh((i_n + b_ih_n) + r * (h_n + b_hh_n))
                        hnr = tmpp.tile([128, NT], F32, tag="hnr", name="hnr")
                        nc.vector.scalar_tensor_tensor(
                            hnr[:], hp[:], bias_sb[:, 10 + oc:11 + oc], r[:],
                            op0=ADD, op1=MULT)
                        npre = tmpp.tile([128, NT], F32, tag="npre", name="npre")
                        nc.vector.scalar_tensor_tensor(
                            npre[:], ip[:], bias_sb[:, 8 + oc:9 + oc], hnr[:],
                            op0=ADD, op1=ADD)
                        nt_ = tmpp.tile([128, NT], F32, tag="nt", name="nt")
                        nc.scalar.activation(nt_[:], npre[:], TANH)
                        # x_new = n + z * (summary - n)
                        d = tmpp.tile([128, NT], F32, tag="d", name="d")
                        nc.vector.tensor_sub(d[:], sT[:, oc, :].bitcast(F32), nt_[:])
                        dz = tmpp.tile([128, NT], F32, tag="dz", name="dz")
                        nc.vector.tensor_mul(dz[:], d[:], z[:])
                        nc.vector.tensor_add(xf_out[:, oc, ts], dz[:], nt_[:])
                    if last:
                        write_output_tile(xf_out, t)
                    else:
                        write_table_tile(xf_out, t)
                if not last:
                    allgather()
                cur = 1 - cur

    nc.compile()
    return nc


def _get_nc(depth: int):
    if depth not in _CACHE:
        _CACHE[depth] = _build(depth)
    return _CACHE[depth]


def _idx_layout(lin):
    """linear int16 idx list (len % 16 == 0) -> [128, len//16] wrapped in 16
    partitions, replicated across the 8 gpsimd core groups."""
    v = lin.reshape(-1, 16).T.astype(np.int16)        # [16, len//16]
    return np.tile(v, (8, 1))                         # [128, len//16]


def _chunk2(w):
    """[256, M] -> [128, 2, M] with [p, c, m] = w[c*128+p, m]."""
    M = w.shape[1]
    return np.ascontiguousarray(w.reshape(2, 128, M).transpose(1, 0, 2))


def prepare_inputs(inputs):
    """host-side preprocessing: returns in_maps for the 8 cores."""
    adj = np.asarray(inputs["nodeAdjacencySpecTensor"]).astype(np.int64)
    names = np.asarray(inputs["nodeNamesEncoded"], dtype=np.float32)
    attrs = np.asarray(inputs["nodeAttributesEncoded"], dtype=np.float32)

    parent = adj[:, 0]
    parent = np.clip(np.where(parent < 0, parent + N, parent), 0, N - 1)
    nbr = adj[:, 1:]
    mask = nbr >= 0
    cnt = np.maximum(mask.sum(1), 1).astype(np.float32)
    safe = np.where(mask, np.clip(nbr, 0, N - 1), ZROW).astype(np.int64)
    inv = (1.0 / cnt).astype(np.float32)

    feat = np.concatenate([names, attrs], axis=1)      # [N, 256] f32

    W_in = np.asarray(inputs["W_in"], np.float32)
    W_p = np.asarray(inputs["W_parent"], np.float32)
    W_n = np.asarray(inputs["W_neighbor"], np.float32)
    W_ih = np.asarray(inputs["W_ih"], np.float32)
    W_hh = np.asarray(inputs["W_hh"], np.float32)
    b_in = np.asarray(inputs["b_in"], np.float32)
    b_p = np.asarray(inputs["b_parent"], np.float32)
    b_n = np.asarray(inputs["b_neighbor"], np.float32)
    b_ih = np.asarray(inputs["b_ih"], np.float32)
    b_hh = np.asarray(inputs["b_hh"], np.float32)

    w_in_a = _chunk2(W_in.T)                            # [128, 2, 256]
    w_ih_a = _chunk2(W_ih.T)                            # [128, 2, 768]
    w_hh_a = _chunk2(W_hh.T)
    w_p_a = _chunk2(W_p.T).astype(BF)
    w_n_a = _chunk2(W_n.T).astype(BF)

    bias = np.zeros((128, 12), np.float32)
    for col, vec in ((0, b_in), (2, b_p + b_n), (4, (b_ih + b_hh)[0:H]),
                     (6, (b_ih + b_hh)[H:2 * H]), (8, b_ih[2 * H:3 * H]),
                     (10, b_hh[2 * H:3 * H])):
        bias[:, col] = vec[0:128]
        bias[:, col + 1] = vec[128:256]

    ident_b = np.eye(128, dtype=BF)
    ident_f = np.eye(128, dtype=np.float32)

    shared = dict(w_in=w_in_a, w_ih=w_ih_a, w_hh=w_hh_a, w_p=w_p_a, w_n=w_n_a,
                  biases=bias, ident_b=ident_b, ident_f=ident_f)

    in_maps = []
    for c in range(NCORES):
        g0 = c * NC_REAL
        # features, transposed + padded
        f = np.zeros((NCP, DIN), np.float32)
        f[:NC_REAL] = feat[g0:g0 + NC_REAL]
        featT_c = np.ascontiguousarray(
            f.T.reshape(2, 128, NCP).transpose(1, 0, 2))
        # inv count broadcast
        iv = np.ones(NCP, np.float32)
        iv[:NC_REAL] = inv[g0:g0 + NC_REAL]
        inv_c = np.broadcast_to(iv, (128, NCP)).copy()
        # indices
        par = np.full(NCP, ZROW, np.int64)
        par[:NC_REAL] = parent[g0:g0 + NC_REAL]
        nbrs = np.full((NCP, K), ZROW, np.int64)
        nbrs[:NC_REAL] = safe[g0:g0 + NC_REAL]
        nbr_t = np.zeros((128, T, NT), np.int16)
        par_t = np.zeros((128, T, NT // 16), np.int16)
        for t in range(T):
            nbr_t[:, t, :] = _idx_layout(nbrs[t * NT:(t + 1) * NT].reshape(-1))
            par_t[:, t, :] = _idx_layout(par[t * NT:(t + 1) * NT])
        in_maps.append(dict(featT=featT_c, invcnt=inv_c, nbr_idx=nbr_t,
                            par_idx=par_t, **shared))
    return in_maps


def run(inputs, trace=False, **kw):
    depth = int(np.asarray(inputs["depth"]))
    nc = _get_nc(depth)
    in_maps = prepare_inputs(inputs)
    res = bass_utils.run_bass_kernel_spmd(nc, in_maps,
                                          core_ids=list(range(NCORES)),
                                          trace=trace, **kw)
    out = np.concatenate([np.asarray(res.results[c]["y"])[:NC_REAL]
                          for c in range(NCORES)], axis=0)
    return np.ascontiguousarray(out.astype(np.float32)), res


def kernel(**inputs) -> np.ndarray:
    out, _ = run(inputs, trace=False)
    return out



# revision 16
# speedup vs baseline: 2.1184x; 2.1184x over previous
"""Trainium2 Bass kernel for nn_NodeInfoPropagate (GNN message passing).

Strategy (8 NeuronCores, node-parallel), v2:
  - Shard the 20000 nodes across 8 cores (2500/core, padded to 2560 = 5 tiles
    of 512).  Weights replicated.  Activations live on-chip transposed
    [feature-on-partition, node-on-free] so matmuls chain without transposes.
  - Gathers: parent rows via dma_gather(transpose=True); neighbor rows via
    dma_gather(transpose=False) whose [lane-on-partition, row-major] output
    feeds 0/1-indicator matmuls on the PE that sum each node's 16 neighbor
    rows (replacing the vector-engine reduce).  All gathers round-robin over
    4 SWDGE queues, which generate descriptors concurrently (~3x).
  - gather commutes with linear maps: p[par] = x[par] @ Wp.T etc., so only
    ONE x table is gathered per layer.  For layer 1 it commutes further
    through W_in: x0[par] = feat[par] @ Win.T + b_in, so layer-1 gathers run
    against the (host-replicated) input feature table starting at t=0 and
    the first AllGather is eliminated.  Layers 2..depth gather a bf16 x
    table AllGathered (RDH) into every core's HBM.
"""

import sys

sys.path.insert(0, "/opt/trn_rl_repo")

import numpy as np
import ml_dtypes

import concourse.bass as bass
import concourse.bacc as bacc
import concourse.tile as tile
import concourse.mybir as mybir
from concourse import bass_utils

N = 20000
K = 16
H = 256
DIN = 256
NCORES = 8
NC_REAL = N // NCORES          # 2500 real nodes per core
NT = 512                       # node tile (matmul free dim / PSUM bank)
T = 5                          # tiles per core
NCP = NT * T                   # 2560 padded nodes per core
NTAB = N + 128                 # table rows (pad keeps alignment comfy)
NQ = (NT // 4) * K             # 2048 neighbor idxs per quarter-tile
AGSPLIT = 3 * NT               # local rows in first AllGather chunk

F32 = mybir.dt.float32
F32R = mybir.dt.float32r
BF16 = mybir.dt.bfloat16
I16 = mybir.dt.int16
BF = ml_dtypes.bfloat16

_CACHE = {}


def _build(depth: int):
    nc = bacc.Bacc("TRN2", target_bir_lowering=False, debug=False,
                   num_devices=NCORES, num_swdge_queues=4)

    feattab = nc.dram_tensor("feattab", [NTAB, DIN], BF16, kind="ExternalInput")
    featT = nc.dram_tensor("featT", [128, 2, NCP], F32, kind="ExternalInput")
    invcnt = nc.dram_tensor("invcnt", [128, NCP], F32, kind="ExternalInput")
    nbr_idx = nc.dram_tensor("nbr_idx", [128, T, NT], I16, kind="ExternalInput")
    par_idx = nc.dram_tensor("par_idx", [128, T, NT // 16], I16, kind="ExternalInput")
    ind_in = nc.dram_tensor("ind_in", [128, T, NT], BF16, kind="ExternalInput")
    w_in = nc.dram_tensor("w_in", [128, 2, H], F32, kind="ExternalInput")
    w_in_b = nc.dram_tensor("w_in_b", [128, 2, H], BF16, kind="ExternalInput")
    w_ih = nc.dram_tensor("w_ih", [128, 2, 3 * H], BF16, kind="ExternalInput")
    w_hh = nc.dram_tensor("w_hh", [128, 2, 3 * H], F32, kind="ExternalInput")
    w_p = nc.dram_tensor("w_p", [128, 2, H], BF16, kind="ExternalInput")
    w_n = nc.dram_tensor("w_n", [128, 2, H], BF16, kind="ExternalInput")
    # bias columns: 0-1 b_in, 2-3 b_p+b_n, 4-5 b_r, 6-7 b_z, 8-9 b_ih_n,
    # 10-11 b_hh_n  (per 128-feature chunk)
    biases = nc.dram_tensor("biases", [128, 12], F32, kind="ExternalInput")
    ident_b = nc.dram_tensor("ident_b", [128, 128], BF16, kind="ExternalInput")
    y = nc.dram_tensor("y", [NCP, H], F32, kind="ExternalOutput")

    SIG = mybir.ActivationFunctionType.Sigmoid
    TANH = mybir.ActivationFunctionType.Tanh
    ADD = mybir.AluOpType.add
    MULT = mybir.AluOpType.mult

    qctr = [0]

    def qn():
        q = qctr[0] % 4
        qctr[0] += 1
        return q

    with tile.TileContext(nc) as tc:
        with (
            tc.tile_pool(name="const", bufs=1) as constp,
            tc.tile_pool(name="state", bufs=1) as statep,
            tc.tile_pool(name="dram", bufs=1, space="DRAM") as dramp,
            tc.tile_pool(name="gpp", bufs=2) as gpp,
            tc.tile_pool(name="gnp", bufs=8) as gnp,
            tc.tile_pool(name="work", bufs=2) as workp,
            tc.tile_pool(name="tmp", bufs=2) as tmpp,
            tc.tile_pool(name="ps", bufs=2, space="PSUM") as psp,
            tc.tile_pool(name="psn", bufs=1, space="PSUM") as psnp,
            tc.tile_pool(name="psg", bufs=4, space="PSUM") as psgp,
        ):
            # ---- resident constants (gather metadata first) -------------
            nbr_sb = constp.tile([128, T, NT], I16, name="nbr_sb")
            nc.sync.dma_start(nbr_sb[:], nbr_idx.ap())
            par_sb = constp.tile([128, T, NT // 16], I16, name="par_sb")
            nc.sync.dma_start(par_sb[:], par_idx.ap())
            ind_sb = constp.tile([128, T, NT], BF16, name="ind_sb")
            nc.sync.dma_start(ind_sb[:], ind_in.ap())
            win_sb = constp.tile([128, 2, H], F32R, name="win_sb")
            nc.sync.dma_start(win_sb[:], w_in.ap().bitcast(F32R))
            winb_sb = constp.tile([128, 2, H], BF16, name="winb_sb")
            nc.sync.dma_start(winb_sb[:], w_in_b.ap())
            wih_sb = constp.tile([128, 2, 3 * H], BF16, name="wih_sb")
            nc.sync.dma_start(wih_sb[:], w_ih.ap())
            whh_sb = constp.tile([128, 2, 3 * H], F32R, name="whh_sb")
            nc.sync.dma_start(whh_sb[:], w_hh.ap().bitcast(F32R))
            wp_sb = constp.tile([128, 2, H], BF16, name="wp_sb")
            nc.sync.dma_start(wp_sb[:], w_p.ap())
            wn_sb = constp.tile([128, 2, H], BF16, name="wn_sb")
            nc.sync.dma_start(wn_sb[:], w_n.ap())
            bias_sb = constp.tile([128, 12], F32, name="bias_sb")
            nc.sync.dma_start(bias_sb[:], biases.ap())
            idb_sb = constp.tile([128, 128], BF16, name="idb_sb")
            nc.sync.dma_start(idb_sb[:], ident_b.ap())
            feat_sb = constp.tile([128, 2, NCP], F32R, name="feat_sb")
            nc.sync.dma_start(feat_sb[:], featT.ap().bitcast(F32R))
            inv_sb = constp.tile([128, NCP], F32, name="inv_sb")
            nc.sync.dma_start(inv_sb[:], invcnt.ap())

            xF = [statep.tile([128, 2, NCP], BF16, name=f"xF{i}") for i in range(2)]

            xloc = dramp.tile([NCP, H], BF16, name="xloc")
            xtab = dramp.tile([NTAB, H], BF16, name="xtab")

            def mm_f32r(out_ps, lhsT, rhs, start, stop):
                nc.tensor.matmul(out_ps, lhsT, rhs, start=start, stop=stop)

            def write_table_tile(xf, t):
                """transpose tile t of xf (bf16) to row-major, DMA to xloc."""
                for b in range(NT // 128):
                    c0 = t * NT + b * 128
                    rm = workp.tile([128, 2, 128], BF16, tag="rm", name="rm")
                    for c in range(2):
                        pst = psp.tile([128, 128], BF16, tag="sum", name="pst")
                        nc.tensor.transpose(pst[:], xf[:, c, c0:c0 + 128],
                                            idb_sb[:])
                        nc.vector.tensor_copy(rm[:, c, :], pst[:])
                    nc.sync.dma_start(xloc[c0:c0 + 128, :], rm[:])

            def write_output_tile(xf, t):
                ts0 = t * NT
                for b in range(NT // 128):
                    rmf = workp.tile([128, 2, 128], F32, tag="rmf", name="rmf")
                    for c in range(2):
                        pst = psp.tile([128, 128], BF16, tag="sum", name="pstf")
                        nc.tensor.transpose(
                            pst[:], xf[:, c, ts0 + b * 128:ts0 + (b + 1) * 128],
                            idb_sb[:])
                        nc.vector.tensor_copy(rmf[:, c, :], pst[:])
                    r0 = ts0 + b * 128
                    nc.sync.dma_start(y[r0:r0 + 128, :], rmf[:])

            xtab_v = xtab[0:N, :].rearrange("(c r) f -> c r f", c=NCORES)

            def allgather(lo, hi):
                if (lo, hi) != (0, NC_REAL):
                    return
                nc.gpsimd.collective_compute(
                    "AllGather", mybir.AluOpType.bypass,
                    replica_groups=[list(range(NCORES))],
                    ins=[xloc[lo:hi, :].opt()],
                    outs=[xtab_v[:, lo:hi, :]],
                )

            def gather_tile(tab, t):
                """parent rows (transposed) + neighbor rows (row-major)."""
                pg = gpp.tile([128, 2, NT], BF16, tag="pgat", name="pg")
                nc.gpsimd.dma_gather(pg[:], tab[:], par_sb[:, t, :],
                                     NT, NT, H, transpose=True,
                                     single_packet=False, queue_num=qn())
                ngs = []
                for qt in range(4):
                    ng = gnp.tile([128, NQ // 128, H], BF16, tag="ng",
                                  name="ng")
                    nc.gpsimd.dma_gather(
                        ng[:], tab[:],
                        nbr_sb[:, t, qt * (NT // 4):(qt + 1) * (NT // 4)],
                        NQ, NQ, H, transpose=False,
                        single_packet=False, queue_num=qn())
                    ngs.append(ng)
                return pg, ngs

            def nbr_sum(t, ngs):
                """0/1-indicator matmuls: sum each node's K neighbor rows.
                Returns two [128, NT] PSUM tiles (feature chunks)."""
                psn = [psnp.tile([128, NT], F32, tag=f"nb{h}", name=f"psn{h}")
                       for h in range(2)]
                for qt, ng in enumerate(ngs):
                    for c in range(NQ // 128):
                        col = qt * (NT // 4) + c * 8
                        for h in range(2):
                            nc.tensor.matmul(
                                psn[h][:, col:col + 8],
                                ng[:, c, h * 128:(h + 1) * 128],
                                ind_sb[:, t, col:col + 8],
                                start=True, stop=True)
                return psn

            def summary_tile(t, par_rhs, nbr_rhs):
                """sT = par_rhs @ Wp.T + nbr_rhs @ Wn.T + (b_p + b_n)."""
                ts = slice(t * NT, (t + 1) * NT)
                sT = workp.tile([128, 2, NT], F32R, tag="sT", name="sT")
                for oc in range(2):
                    ps = psp.tile([128, NT], F32, tag="sum", name="psS")
                    for hc in range(2):
                        nc.tensor.matmul(ps[:],
                                         wp_sb[:, hc, oc * 128:(oc + 1) * 128],
                                         par_rhs[:, hc, :],
                                         start=(hc == 0), stop=False)
                    for hc in range(2):
                        nc.tensor.matmul(ps[:],
                                         wn_sb[:, hc, oc * 128:(oc + 1) * 128],
                                         nbr_rhs[:, hc, :],
                                         start=False, stop=(hc == 1))
                    nc.vector.tensor_scalar_add(sT[:, oc, :], ps[:],
                                                bias_sb[:, 2 + oc:3 + oc])
                return sT

            def gru_tile(t, xf_in, xf_out, sT):
                ts = slice(t * NT, (t + 1) * NT)
                for oc in range(2):
                    rp = psgp.tile([128, NT], F32, tag="gate", name="rp")
                    zp = psgp.tile([128, NT], F32, tag="gate", name="zp")
                    ip = psgp.tile([128, NT], F32, tag="gate", name="ip")
                    hp = psgp.tile([128, NT], F32, tag="gate", name="hp")
                    for gate, pst in ((0, rp), (1, zp)):
                        o0 = gate * H + oc * 128
                        for hc in range(2):
                            mm_f32r(pst[:], wih_sb[:, hc, o0:o0 + 128],
                                    xf_in[:, hc, ts],
                                    start=(hc == 0), stop=False)
                        for hc in range(2):
                            mm_f32r(pst[:], whh_sb[:, hc, o0:o0 + 128],
                                    sT[:, hc, :],
                                    start=False, stop=(hc == 1))
                    o0 = 2 * H + oc * 128
                    for hc in range(2):
                        mm_f32r(ip[:], wih_sb[:, hc, o0:o0 + 128],
                                xf_in[:, hc, ts],
                                start=(hc == 0), stop=(hc == 1))
                    for hc in range(2):
                        mm_f32r(hp[:], whh_sb[:, hc, o0:o0 + 128],
                                sT[:, hc, :],
                                start=(hc == 0), stop=(hc == 1))
                    r = tmpp.tile([128, NT], F32, tag="r", name="r")
                    nc.scalar.activation(r[:], rp[:], SIG,
                                         bias=bias_sb[:, 4 + oc:5 + oc])
                    z = tmpp.tile([128, NT], F32, tag="z", name="z")
                    nc.scalar.activation(z[:], zp[:], SIG,
                                         bias=bias_sb[:, 6 + oc:7 + oc])
                    # n = tanh((i_n + b_ih_n) + r * (h_n + b_hh_n))
                    hnr = tmpp.tile([128, NT], F32, tag="hnr", name="hnr")
                    nc.vector.scalar_tensor_tensor(
                        hnr[:], hp[:], bias_sb[:, 10 + oc:11 + oc], r[:],
                        op0=ADD, op1=MULT)
                    npre = tmpp.tile([128, NT], F32, tag="npre", name="npre")
                    nc.vector.scalar_tensor_tensor(
                        npre[:], ip[:], bias_sb[:, 8 + oc:9 + oc], hnr[:],
                        op0=ADD, op1=ADD)
                    nt_ = tmpp.tile([128, NT], F32, tag="nt", name="nt")
                    nc.scalar.activation(nt_[:], npre[:], TANH)
                    # x_new = n + z * (summary - n)
                    d = tmpp.tile([128, NT], F32, tag="d", name="d")
                    nc.vector.tensor_sub(d[:], sT[:, oc, :].bitcast(F32), nt_[:])
                    dz = tmpp.tile([128, NT], F32, tag="dz", name="dz")
                    nc.vector.tensor_mul(dz[:], d[:], z[:])
                    nc.vector.tensor_add(xf_out[:, oc, ts], dz[:], nt_[:])

            # ---- layer 0: x0 = W_in @ feat + b_in  (local, f32) ---------
            for t in range(T):
                ts = slice(t * NT, (t + 1) * NT)
                for oc in range(2):
                    ps = psp.tile([128, NT], F32, tag="sum", name="ps0")
                    for dc in range(2):
                        mm_f32r(ps[:], win_sb[:, dc, oc * 128:(oc + 1) * 128],
                                feat_sb[:, dc, ts], start=(dc == 0), stop=(dc == 1))
                    nc.vector.tensor_scalar_add(xF[0][:, oc, ts], ps[:],
                                                bias_sb[:, oc:oc + 1])
                if depth == 0:
                    write_output_tile(xF[0], t)

            # ---- GRU layers ---------------------------------------------
            cur = 0
            for layer in range(depth):
                first = layer == 0
                last = layer == depth - 1
                xf_in, xf_out = xF[cur], xF[1 - cur]
                for t in range(T):
                    ts = slice(t * NT, (t + 1) * NT)
                    if first:
                        # gather FEAT rows; x0[g] = feat[g] @ Win.T + b_in
                        pg, ngs = gather_tile(feattab, t)
                        psn = nbr_sum(t, ngs)
                        fng = workp.tile([128, 2, NT], BF16, tag="fng",
                                         name="fng")
                        for h in range(2):
                            nc.vector.tensor_mul(fng[:, h, :], psn[h][:],
                                                 inv_sb[:, ts])
                        x0n = workp.tile([128, 2, NT], BF16, tag="x0n",
                                         name="x0n")
                        x0p = workp.tile([128, 2, NT], BF16, tag="x0p",
                                         name="x0p")
                        for dst, rhs in ((x0n, fng), (x0p, pg)):
                            for oc in range(2):
                                ps = psp.tile([128, NT], F32, tag="sum",
                                              name="psW")
                                for dc in range(2):
                                    nc.tensor.matmul(
                                        ps[:],
                                        winb_sb[:, dc, oc * 128:(oc + 1) * 128],
                                        rhs[:, dc, :],
                                        start=(dc == 0), stop=(dc == 1))
                                nc.vector.tensor_scalar_add(
                                    dst[:, oc, :], ps[:], bias_sb[:, oc:oc + 1])
                        sT = summary_tile(t, x0p, x0n)
                    else:
                        pg, ngs = gather_tile(xtab, t)
                        psn = nbr_sum(t, ngs)
                        nmean = workp.tile([128, 2, NT], BF16, tag="fng",
                                           name="nmean")
                        for h in range(2):
                            nc.vector.tensor_mul(nmean[:, h, :], psn[h][:],
                                                 inv_sb[:, ts])
                        sT = summary_tile(t, pg, nmean)
                    gru_tile(t, xf_in, xf_out, sT)
                    if last:
                        write_output_tile(xf_out, t)
                    else:
                        write_table_tile(xf_out, t)
                if not last:
                    allgather(0, NC_REAL)
                cur = 1 - cur

    nc.compile()
    return nc


def _get_nc(depth: int):
    if depth not in _CACHE:
        _CACHE[depth] = _build(depth)
    return _CACHE[depth]


def _idx_layout(lin):
    """linear int16 idx list (len % 16 == 0) -> [128, len//16] wrapped in 16
    partitions, replicated across the 8 gpsimd core groups."""
    v = lin.reshape(-1, 16).T.astype(np.int16)        # [16, len//16]
    return np.tile(v, (8, 1))                         # [128, len//16]


def _chunk2(w):
    """[256, M] -> [128, 2, M] with [p, c, m] = w[c*128+p, m]."""
    M = w.shape[1]
    return np.ascontiguousarray(w.reshape(2, 128, M).transpose(1, 0, 2))


def prepare_inputs(inputs):
    """host-side preprocessing: returns in_maps for the 8 cores."""
    adj = np.asarray(inputs["nodeAdjacencySpecTensor"]).astype(np.int64)
    names = np.asarray(inputs["nodeNamesEncoded"], dtype=np.float32)
    attrs = np.asarray(inputs["nodeAttributesEncoded"], dtype=np.float32)

    parent = adj[:, 0]
    parent = np.clip(np.where(parent < 0, parent + N, parent), 0, N - 1)
    nbr = adj[:, 1:]
    mask = nbr >= 0
    cnt = np.maximum(mask.sum(1), 1).astype(np.float32)
    safe = np.where(mask, np.clip(nbr, 0, N - 1), 0).astype(np.int64)
    inv = (1.0 / cnt).astype(np.float32)

    feat = np.concatenate([names, attrs], axis=1)      # [N, 256] f32
    feattab = np.zeros((NTAB, DIN), dtype=BF)
    feattab[:N] = feat.astype(BF)

    W_in = np.asarray(inputs["W_in"], np.float32)
    W_p = np.asarray(inputs["W_parent"], np.float32)
    W_n = np.asarray(inputs["W_neighbor"], np.float32)
    W_ih = np.asarray(inputs["W_ih"], np.float32)
    W_hh = np.asarray(inputs["W_hh"], np.float32)
    b_in = np.asarray(inputs["b_in"], np.float32)
    b_p = np.asarray(inputs["b_parent"], np.float32)
    b_n = np.asarray(inputs["b_neighbor"], np.float32)
    b_ih = np.asarray(inputs["b_ih"], np.float32)
    b_hh = np.asarray(inputs["b_hh"], np.float32)

    w_in_a = _chunk2(W_in.T)                            # [128, 2, 256]
    w_ih_a = _chunk2(W_ih.T).astype(BF)                 # [128, 2, 768]
    w_hh_a = _chunk2(W_hh.T)
    w_p_a = _chunk2(W_p.T).astype(BF)
    w_n_a = _chunk2(W_n.T).astype(BF)

    bias = np.zeros((128, 12), np.float32)
    for col, vec in ((0, b_in), (2, b_p + b_n), (4, (b_ih + b_hh)[0:H]),
                     (6, (b_ih + b_hh)[H:2 * H]), (8, b_ih[2 * H:3 * H]),
                     (10, b_hh[2 * H:3 * H])):
        bias[:, col] = vec[0:128]
        bias[:, col + 1] = vec[128:256]

    ident_b = np.eye(128, dtype=BF)

    shared = dict(feattab=feattab, w_in=w_in_a,
                  w_in_b=w_in_a.astype(BF), w_ih=w_ih_a, w_hh=w_hh_a,
                  w_p=w_p_a, w_n=w_n_a, biases=bias, ident_b=ident_b)

    # indicator row for (node-in-tile n, slot k): partition (n%8)*16 + k
    ind_rows = (np.arange(NT)[:, None] % 8) * 16 + np.arange(K)[None, :]

    in_maps = []
    for c in range(NCORES):
        g0 = c * NC_REAL
        # features, transposed + padded
        f = np.zeros((NCP, DIN), np.float32)
        f[:NC_REAL] = feat[g0:g0 + NC_REAL]
        featT_c = np.ascontiguousarray(
            f.T.reshape(2, 128, NCP).transpose(1, 0, 2))
        # inv count broadcast
        iv = np.ones(NCP, np.float32)
        iv[:NC_REAL] = inv[g0:g0 + NC_REAL]
        inv_c = np.broadcast_to(iv, (128, NCP)).copy()
        # indices (all clipped to valid rows; masking via indicator)
        par = np.zeros(NCP, np.int64)
        par[:NC_REAL] = parent[g0:g0 + NC_REAL]
        nbrs = np.zeros((NCP, K), np.int64)
        nbrs[:NC_REAL] = safe[g0:g0 + NC_REAL]
        msk = np.zeros((NCP, K), np.float32)
        msk[:NC_REAL] = mask[g0:g0 + NC_REAL]
        nbr_t = np.zeros((128, T, NT), np.int16)
        par_t = np.zeros((128, T, NT // 16), np.int16)
        ind_t = np.zeros((128, T, NT), dtype=BF)
        for t in range(T):
            nbr_t[:, t, :] = _idx_layout(nbrs[t * NT:(t + 1) * NT].reshape(-1))
            par_t[:, t, :] = _idx_layout(par[t * NT:(t + 1) * NT])
            m = msk[t * NT:(t + 1) * NT]               # [NT, K]
            M = np.zeros((128, NT), np.float32)
            M[ind_rows.ravel(), np.repeat(np.arange(NT), K)] = m.ravel()
            ind_t[:, t, :] = M.astype(BF)
        in_maps.append(dict(featT=featT_c, invcnt=inv_c, nbr_idx=nbr_t,
                            par_idx=par_t, ind_in=ind_t, **shared))
    return in_maps


def run(inputs, trace=False, **kw):
    depth = int(np.asarray(inputs["depth"]))
    nc = _get_nc(depth)
    in_maps = prepare_inputs(inputs)
    res = bass_utils.run_bass_kernel_spmd(nc, in_maps,
                                          core_ids=list(range(NCORES)),
                                          trace=trace, **kw)
    out = np.concatenate([np.asarray(res.results[c]["y"])[:NC_REAL]
                          for c in range(NCORES)], axis=0)
    return np.ascontiguousarray(out.astype(np.float32)), res


def kernel(**inputs) -> np.ndarray:
    out, _ = run(inputs, trace=False)
    return out


# revision 17
# speedup vs baseline: 2.2311x; 1.0532x over previous
"""Trainium2 Bass kernel for nn_NodeInfoPropagate (GNN message passing).

Strategy (8 NeuronCores, node-parallel), v2:
  - Shard the 20000 nodes across 8 cores (2500/core, padded to 2560 = 5 tiles
    of 512).  Weights replicated.  Activations live on-chip transposed
    [feature-on-partition, node-on-free] so matmuls chain without transposes.
  - Gathers: parent rows via dma_gather(transpose=True); neighbor rows via
    dma_gather(transpose=False) whose [lane-on-partition, row-major] output
    feeds 0/1-indicator matmuls on the PE that sum each node's 16 neighbor
    rows (replacing the vector-engine reduce).  All gathers round-robin over
    4 SWDGE queues, which generate descriptors concurrently (~3x).
  - gather commutes with linear maps: p[par] = x[par] @ Wp.T etc., so only
    ONE x table is gathered per layer.  For layer 1 it commutes further
    through W_in: x0[par] = feat[par] @ Win.T + b_in, so layer-1 gathers run
    against the (host-replicated) input feature table starting at t=0 and
    the first AllGather is eliminated.  Layers 2..depth gather a bf16 x
    table AllGathered (RDH) into every core's HBM.
"""

import sys

sys.path.insert(0, "/opt/trn_rl_repo")

import numpy as np
import ml_dtypes

import concourse.bass as bass
import concourse.bacc as bacc
import concourse.tile as tile
import concourse.mybir as mybir
from concourse import bass_utils

N = 20000
K = 16
H = 256
DIN = 256
NCORES = 8
NC_REAL = N // NCORES          # 2500 real nodes per core
NT = 512                       # node tile (matmul free dim / PSUM bank)
T = 5                          # tiles per core
NCP = NT * T                   # 2560 padded nodes per core
NTAB = N + 128                 # table rows (pad keeps alignment comfy)
NQ = (NT // 4) * K             # 2048 neighbor idxs per quarter-tile
AGSPLIT = 3 * NT               # local rows in first AllGather chunk

F32 = mybir.dt.float32
F32R = mybir.dt.float32r
BF16 = mybir.dt.bfloat16
I16 = mybir.dt.int16
BF = ml_dtypes.bfloat16

_CACHE = {}


def _build(depth: int):
    nc = bacc.Bacc("TRN2", target_bir_lowering=False, debug=False,
                   num_devices=NCORES, num_swdge_queues=4)

    feattab = nc.dram_tensor("feattab", [NTAB, DIN], BF16, kind="ExternalInput")
    featT = nc.dram_tensor("featT", [128, 2, NCP], F32, kind="ExternalInput")
    invcnt = nc.dram_tensor("invcnt", [128, NCP], F32, kind="ExternalInput")
    nbr_idx = nc.dram_tensor("nbr_idx", [128, T, NT], I16, kind="ExternalInput")
    par_idx = nc.dram_tensor("par_idx", [128, T, NT // 16], I16, kind="ExternalInput")
    ind_in = nc.dram_tensor("ind_in", [128, T, NT], BF16, kind="ExternalInput")
    w_in = nc.dram_tensor("w_in", [128, 2, H], F32, kind="ExternalInput")
    w_in_b = nc.dram_tensor("w_in_b", [128, 2, H], BF16, kind="ExternalInput")
    w_ih = nc.dram_tensor("w_ih", [128, 2, 3 * H], BF16, kind="ExternalInput")
    w_hh = nc.dram_tensor("w_hh", [128, 2, 3 * H], F32, kind="ExternalInput")
    w_p = nc.dram_tensor("w_p", [128, 2, H], BF16, kind="ExternalInput")
    w_n = nc.dram_tensor("w_n", [128, 2, H], BF16, kind="ExternalInput")
    # bias columns: 0-1 b_in, 2-3 b_p+b_n, 4-5 b_r, 6-7 b_z, 8-9 b_ih_n,
    # 10-11 b_hh_n  (per 128-feature chunk)
    biases = nc.dram_tensor("biases", [128, 12], F32, kind="ExternalInput")
    ident_b = nc.dram_tensor("ident_b", [128, 128], BF16, kind="ExternalInput")
    y = nc.dram_tensor("y", [NCP, H], F32, kind="ExternalOutput")

    SIG = mybir.ActivationFunctionType.Sigmoid
    TANH = mybir.ActivationFunctionType.Tanh
    ADD = mybir.AluOpType.add
    MULT = mybir.AluOpType.mult

    qctr = [0]

    def qn():
        q = qctr[0] % 4
        qctr[0] += 1
        return q

    with tile.TileContext(nc) as tc:
        with (
            tc.tile_pool(name="const", bufs=1) as constp,
            tc.tile_pool(name="state", bufs=1) as statep,
            tc.tile_pool(name="dram", bufs=1, space="DRAM") as dramp,
            tc.tile_pool(name="gpp", bufs=2) as gpp,
            tc.tile_pool(name="gnp", bufs=8) as gnp,
            tc.tile_pool(name="work", bufs=2) as workp,
            tc.tile_pool(name="tmp", bufs=2) as tmpp,
            tc.tile_pool(name="ps", bufs=2, space="PSUM") as psp,
            tc.tile_pool(name="psn", bufs=1, space="PSUM") as psnp,
            tc.tile_pool(name="psg", bufs=4, space="PSUM") as psgp,
        ):
            # ---- resident constants (gather metadata first) -------------
            nbr_sb = constp.tile([128, T, NT], I16, name="nbr_sb")
            nc.sync.dma_start(nbr_sb[:], nbr_idx.ap())
            par_sb = constp.tile([128, T, NT // 16], I16, name="par_sb")
            nc.sync.dma_start(par_sb[:], par_idx.ap())
            ind_sb = constp.tile([128, T, NT], BF16, name="ind_sb")
            nc.sync.dma_start(ind_sb[:], ind_in.ap())
            win_sb = constp.tile([128, 2, H], F32R, name="win_sb")
            nc.sync.dma_start(win_sb[:], w_in.ap().bitcast(F32R))
            winb_sb = constp.tile([128, 2, H], BF16, name="winb_sb")
            nc.sync.dma_start(winb_sb[:], w_in_b.ap())
            wih_sb = constp.tile([128, 2, 3 * H], BF16, name="wih_sb")
            nc.sync.dma_start(wih_sb[:], w_ih.ap())
            whh_sb = constp.tile([128, 2, 3 * H], F32R, name="whh_sb")
            nc.sync.dma_start(whh_sb[:], w_hh.ap().bitcast(F32R))
            wp_sb = constp.tile([128, 2, H], BF16, name="wp_sb")
            nc.sync.dma_start(wp_sb[:], w_p.ap())
            wn_sb = constp.tile([128, 2, H], BF16, name="wn_sb")
            nc.sync.dma_start(wn_sb[:], w_n.ap())
            bias_sb = constp.tile([128, 12], F32, name="bias_sb")
            nc.sync.dma_start(bias_sb[:], biases.ap())
            idb_sb = constp.tile([128, 128], BF16, name="idb_sb")
            nc.sync.dma_start(idb_sb[:], ident_b.ap())
            feat_sb = constp.tile([128, 2, NCP], F32R, name="feat_sb")
            nc.sync.dma_start(feat_sb[:], featT.ap().bitcast(F32R))
            inv_sb = constp.tile([128, NCP], F32, name="inv_sb")
            nc.sync.dma_start(inv_sb[:], invcnt.ap())

            xF = [statep.tile([128, 2, NCP], BF16, name=f"xF{i}") for i in range(2)]

            xloc = dramp.tile([NCP, H], BF16, name="xloc")
            xtab = dramp.tile([NTAB, H], BF16, name="xtab")

            def mm_f32r(out_ps, lhsT, rhs, start, stop):
                nc.tensor.matmul(out_ps, lhsT, rhs, start=start, stop=stop)

            def write_table_tile(xf, t):
                """transpose tile t of xf (bf16) to row-major, DMA to xloc."""
                for b in range(NT // 128):
                    c0 = t * NT + b * 128
                    rm = workp.tile([128, 2, 128], BF16, tag="rm", name="rm")
                    for c in range(2):
                        pst = psp.tile([128, 128], BF16, tag="sum", name="pst")
                        nc.tensor.transpose(pst[:], xf[:, c, c0:c0 + 128],
                                            idb_sb[:])
                        nc.vector.tensor_copy(rm[:, c, :], pst[:])
                    nc.sync.dma_start(xloc[c0:c0 + 128, :], rm[:])

            def write_output_tile(xf, t):
                ts0 = t * NT
                for b in range(NT // 128):
                    rmf = workp.tile([128, 2, 128], F32, tag="rmf", name="rmf")
                    for c in range(2):
                        pst = psp.tile([128, 128], BF16, tag="sum", name="pstf")
                        nc.tensor.transpose(
                            pst[:], xf[:, c, ts0 + b * 128:ts0 + (b + 1) * 128],
                            idb_sb[:])
                        nc.vector.tensor_copy(rmf[:, c, :], pst[:])
                    r0 = ts0 + b * 128
                    nc.sync.dma_start(y[r0:r0 + 128, :], rmf[:])

            def allgather(lo, hi):
                if (lo, hi) != (0, NC_REAL):
                    return
                nc.gpsimd.collective_compute(
                    "AllGather", mybir.AluOpType.bypass,
                    replica_groups=[list(range(NCORES))],
                    ins=[xloc[0:NC_REAL, :].opt()],
                    outs=[xtab[0:N, :].opt()],
                )

            def gather_tile(tab, t):
                """parent rows (transposed) + neighbor rows (row-major)."""
                pg = gpp.tile([128, 2, NT], BF16, tag="pgat", name="pg")
                nc.gpsimd.dma_gather(pg[:], tab[:], par_sb[:, t, :],
                                     NT, NT, H, transpose=True,
                                     single_packet=False, queue_num=qn())
                ngs = []
                for qt in range(4):
                    ng = gnp.tile([128, NQ // 128, H], BF16, tag="ng",
                                  name="ng")
                    nc.gpsimd.dma_gather(
                        ng[:], tab[:],
                        nbr_sb[:, t, qt * (NT // 4):(qt + 1) * (NT // 4)],
                        NQ, NQ, H, transpose=False,
                        single_packet=False, queue_num=qn())
                    ngs.append(ng)
                return pg, ngs

            def nbr_sum(t, ngs):
                """0/1-indicator matmuls: sum each node's K neighbor rows.
                Returns two [128, NT] PSUM tiles (feature chunks)."""
                psn = [psnp.tile([128, NT], F32, tag=f"nb{h}", name=f"psn{h}")
                       for h in range(2)]
                for qt, ng in enumerate(ngs):
                    for c in range(NQ // 128):
                        col = qt * (NT // 4) + c * 8
                        for h in range(2):
                            nc.tensor.matmul(
                                psn[h][:, col:col + 8],
                                ng[:, c, h * 128:(h + 1) * 128],
                                ind_sb[:, t, col:col + 8],
                                start=True, stop=True)
                return psn

            def summary_tile(t, par_rhs, nbr_rhs):
                """sT = par_rhs @ Wp.T + nbr_rhs @ Wn.T + (b_p + b_n)."""
                ts = slice(t * NT, (t + 1) * NT)
                sT = workp.tile([128, 2, NT], F32R, tag="sT", name="sT")
                for oc in range(2):
                    ps = psp.tile([128, NT], F32, tag="sum", name="psS")
                    for hc in range(2):
                        nc.tensor.matmul(ps[:],
                                         wp_sb[:, hc, oc * 128:(oc + 1) * 128],
                                         par_rhs[:, hc, :],
                                         start=(hc == 0), stop=False)
                    for hc in range(2):
                        nc.tensor.matmul(ps[:],
                                         wn_sb[:, hc, oc * 128:(oc + 1) * 128],
                                         nbr_rhs[:, hc, :],
                                         start=False, stop=(hc == 1))
                    nc.vector.tensor_scalar_add(sT[:, oc, :], ps[:],
                                                bias_sb[:, 2 + oc:3 + oc])
                return sT

            def gru_tile(t, xf_in, xf_out, sT):
                ts = slice(t * NT, (t + 1) * NT)
                for oc in range(2):
                    rp = psgp.tile([128, NT], F32, tag="gate", name="rp")
                    zp = psgp.tile([128, NT], F32, tag="gate", name="zp")
                    ip = psgp.tile([128, NT], F32, tag="gate", name="ip")
                    hp = psgp.tile([128, NT], F32, tag="gate", name="hp")
                    for gate, pst in ((0, rp), (1, zp)):
                        o0 = gate * H + oc * 128
                        for hc in range(2):
                            mm_f32r(pst[:], wih_sb[:, hc, o0:o0 + 128],
                                    xf_in[:, hc, ts],
                                    start=(hc == 0), stop=False)
                        for hc in range(2):
                            mm_f32r(pst[:], whh_sb[:, hc, o0:o0 + 128],
                                    sT[:, hc, :],
                                    start=False, stop=(hc == 1))
                    o0 = 2 * H + oc * 128
                    for hc in range(2):
                        mm_f32r(ip[:], wih_sb[:, hc, o0:o0 + 128],
                                xf_in[:, hc, ts],
                                start=(hc == 0), stop=(hc == 1))
                    for hc in range(2):
                        mm_f32r(hp[:], whh_sb[:, hc, o0:o0 + 128],
                                sT[:, hc, :],
                                start=(hc == 0), stop=(hc == 1))
                    r = tmpp.tile([128, NT], F32, tag="r", name="r")
                    nc.scalar.activation(r[:], rp[:], SIG,
                                         bias=bias_sb[:, 4 + oc:5 + oc])
                    z = tmpp.tile([128, NT], F32, tag="z", name="z")
                    nc.scalar.activation(z[:], zp[:], SIG,
                                         bias=bias_sb[:, 6 + oc:7 + oc])
                    # n = tanh((i_n + b_ih_n) + r * (h_n + b_hh_n))
                    hnr = tmpp.tile([128, NT], F32, tag="hnr", name="hnr")
                    nc.vector.scalar_tensor_tensor(
                        hnr[:], hp[:], bias_sb[:, 10 + oc:11 + oc], r[:],
                        op0=ADD, op1=MULT)
                    npre = tmpp.tile([128, NT], F32, tag="npre", name="npre")
                    nc.vector.scalar_tensor_tensor(
                        npre[:], ip[:], bias_sb[:, 8 + oc:9 + oc], hnr[:],
                        op0=ADD, op1=ADD)
                    nt_ = tmpp.tile([128, NT], F32, tag="nt", name="nt")
                    nc.scalar.activation(nt_[:], npre[:], TANH)
                    # x_new = n + z * (summary - n)
                    d = tmpp.tile([128, NT], F32, tag="d", name="d")
                    nc.vector.tensor_sub(d[:], sT[:, oc, :].bitcast(F32), nt_[:])
                    dz = tmpp.tile([128, NT], F32, tag="dz", name="dz")
                    nc.vector.tensor_mul(dz[:], d[:], z[:])
                    nc.vector.tensor_add(xf_out[:, oc, ts], dz[:], nt_[:])

            # ---- layer 0: x0 = W_in @ feat + b_in  (local, f32) ---------
            for t in range(T):
                ts = slice(t * NT, (t + 1) * NT)
                for oc in range(2):
                    ps = psp.tile([128, NT], F32, tag="sum", name="ps0")
                    for dc in range(2):
                        mm_f32r(ps[:], win_sb[:, dc, oc * 128:(oc + 1) * 128],
                                feat_sb[:, dc, ts], start=(dc == 0), stop=(dc == 1))
                    nc.vector.tensor_scalar_add(xF[0][:, oc, ts], ps[:],
                                                bias_sb[:, oc:oc + 1])
                if depth == 0:
                    write_output_tile(xF[0], t)

            # ---- GRU layers ---------------------------------------------
            cur = 0
            for layer in range(depth):
                first = layer == 0
                last = layer == depth - 1
                xf_in, xf_out = xF[cur], xF[1 - cur]
                for t in range(T):
                    ts = slice(t * NT, (t + 1) * NT)
                    if first:
                        # gather FEAT rows; x0[g] = feat[g] @ Win.T + b_in
                        pg, ngs = gather_tile(feattab, t)
                        psn = nbr_sum(t, ngs)
                        fng = workp.tile([128, 2, NT], BF16, tag="fng",
                                         name="fng")
                        for h in range(2):
                            nc.vector.tensor_mul(fng[:, h, :], psn[h][:],
                                                 inv_sb[:, ts])
                        x0n = workp.tile([128, 2, NT], BF16, tag="x0n",
                                         name="x0n")
                        x0p = workp.tile([128, 2, NT], BF16, tag="x0p",
                                         name="x0p")
                        for dst, rhs in ((x0n, fng), (x0p, pg)):
                            for oc in range(2):
                                ps = psp.tile([128, NT], F32, tag="sum",
                                              name="psW")
                                for dc in range(2):
                                    nc.tensor.matmul(
                                        ps[:],
                                        winb_sb[:, dc, oc * 128:(oc + 1) * 128],
                                        rhs[:, dc, :],
                                        start=(dc == 0), stop=(dc == 1))
                                nc.vector.tensor_scalar_add(
                                    dst[:, oc, :], ps[:], bias_sb[:, oc:oc + 1])
                        sT = summary_tile(t, x0p, x0n)
                    else:
                        pg, ngs = gather_tile(xtab, t)
                        psn = nbr_sum(t, ngs)
                        nmean = workp.tile([128, 2, NT], BF16, tag="fng",
                                           name="nmean")
                        for h in range(2):
                            nc.vector.tensor_mul(nmean[:, h, :], psn[h][:],
                                                 inv_sb[:, ts])
                        sT = summary_tile(t, pg, nmean)
                    gru_tile(t, xf_in, xf_out, sT)
                    if last:
                        write_output_tile(xf_out, t)
                    else:
                        write_table_tile(xf_out, t)
                if not last:
                    allgather(0, NC_REAL)
                cur = 1 - cur

    nc.compile()
    return nc


def _get_nc(depth: int):
    if depth not in _CACHE:
        _CACHE[depth] = _build(depth)
    return _CACHE[depth]


def _idx_layout(lin):
    """linear int16 idx list (len % 16 == 0) -> [128, len//16] wrapped in 16
    partitions, replicated across the 8 gpsimd core groups."""
    v = lin.reshape(-1, 16).T.astype(np.int16)        # [16, len//16]
    return np.tile(v, (8, 1))                         # [128, len//16]


def _chunk2(w):
    """[256, M] -> [128, 2, M] with [p, c, m] = w[c*128+p, m]."""
    M = w.shape[1]
    return np.ascontiguousarray(w.reshape(2, 128, M).transpose(1, 0, 2))


def prepare_inputs(inputs):
    """host-side preprocessing: returns in_maps for the 8 cores."""
    adj = np.asarray(inputs["nodeAdjacencySpecTensor"]).astype(np.int64)
    names = np.asarray(inputs["nodeNamesEncoded"], dtype=np.float32)
    attrs = np.asarray(inputs["nodeAttributesEncoded"], dtype=np.float32)

    parent = adj[:, 0]
    parent = np.clip(np.where(parent < 0, parent + N, parent), 0, N - 1)
    nbr = adj[:, 1:]
    mask = nbr >= 0
    cnt = np.maximum(mask.sum(1), 1).astype(np.float32)
    safe = np.where(mask, np.clip(nbr, 0, N - 1), 0).astype(np.int64)
    inv = (1.0 / cnt).astype(np.float32)

    feat = np.concatenate([names, attrs], axis=1)      # [N, 256] f32
    feattab = np.zeros((NTAB, DIN), dtype=BF)
    feattab[:N] = feat.astype(BF)

    W_in = np.asarray(inputs["W_in"], np.float32)
    W_p = np.asarray(inputs["W_parent"], np.float32)
    W_n = np.asarray(inputs["W_neighbor"], np.float32)
    W_ih = np.asarray(inputs["W_ih"], np.float32)
    W_hh = np.asarray(inputs["W_hh"], np.float32)
    b_in = np.asarray(inputs["b_in"], np.float32)
    b_p = np.asarray(inputs["b_parent"], np.float32)
    b_n = np.asarray(inputs["b_neighbor"], np.float32)
    b_ih = np.asarray(inputs["b_ih"], np.float32)
    b_hh = np.asarray(inputs["b_hh"], np.float32)

    w_in_a = _chunk2(W_in.T)                            # [128, 2, 256]
    w_ih_a = _chunk2(W_ih.T).astype(BF)                 # [128, 2, 768]
    w_hh_a = _chunk2(W_hh.T)
    w_p_a = _chunk2(W_p.T).astype(BF)
    w_n_a = _chunk2(W_n.T).astype(BF)

    bias = np.zeros((128, 12), np.float32)
    for col, vec in ((0, b_in), (2, b_p + b_n), (4, (b_ih + b_hh)[0:H]),
                     (6, (b_ih + b_hh)[H:2 * H]), (8, b_ih[2 * H:3 * H]),
                     (10, b_hh[2 * H:3 * H])):
        bias[:, col] = vec[0:128]
        bias[:, col + 1] = vec[128:256]

    ident_b = np.eye(128, dtype=BF)

    shared = dict(feattab=feattab, w_in=w_in_a,
                  w_in_b=w_in_a.astype(BF), w_ih=w_ih_a, w_hh=w_hh_a,
                  w_p=w_p_a, w_n=w_n_a, biases=bias, ident_b=ident_b)

    # indicator row for (node-in-tile n, slot k): partition (n%8)*16 + k
    ind_rows = (np.arange(NT)[:, None] % 8) * 16 + np.arange(K)[None, :]

    in_maps = []
    for c in range(NCORES):
        g0 = c * NC_REAL
        # features, transposed + padded
        f = np.zeros((NCP, DIN), np.float32)
        f[:NC_REAL] = feat[g0:g0 + NC_REAL]
        featT_c = np.ascontiguousarray(
            f.T.reshape(2, 128, NCP).transpose(1, 0, 2))
        # inv count broadcast
        iv = np.ones(NCP, np.float32)
        iv[:NC_REAL] = inv[g0:g0 + NC_REAL]
        inv_c = np.broadcast_to(iv, (128, NCP)).copy()
        # indices (all clipped to valid rows; masking via indicator)
        par = np.zeros(NCP, np.int64)
        par[:NC_REAL] = parent[g0:g0 + NC_REAL]
        nbrs = np.zeros((NCP, K), np.int64)
        nbrs[:NC_REAL] = safe[g0:g0 + NC_REAL]
        msk = np.zeros((NCP, K), np.float32)
        msk[:NC_REAL] = mask[g0:g0 + NC_REAL]
        nbr_t = np.zeros((128, T, NT), np.int16)
        par_t = np.zeros((128, T, NT // 16), np.int16)
        ind_t = np.zeros((128, T, NT), dtype=BF)
        for t in range(T):
            nbr_t[:, t, :] = _idx_layout(nbrs[t * NT:(t + 1) * NT].reshape(-1))
            par_t[:, t, :] = _idx_layout(par[t * NT:(t + 1) * NT])
            m = msk[t * NT:(t + 1) * NT]               # [NT, K]
            M = np.zeros((128, NT), np.float32)
            M[ind_rows.ravel(), np.repeat(np.arange(NT), K)] = m.ravel()
            ind_t[:, t, :] = M.astype(BF)
        in_maps.append(dict(featT=featT_c, invcnt=inv_c, nbr_idx=nbr_t,
                            par_idx=par_t, ind_in=ind_t, **shared))
    return in_maps


def run(inputs, trace=False, **kw):
    depth = int(np.asarray(inputs["depth"]))
    nc = _get_nc(depth)
    in_maps = prepare_inputs(inputs)
    res = bass_utils.run_bass_kernel_spmd(nc, in_maps,
                                          core_ids=list(range(NCORES)),
                                          trace=trace, **kw)
    out = np.concatenate([np.asarray(res.results[c]["y"])[:NC_REAL]
                          for c in range(NCORES)], axis=0)
    return np.ascontiguousarray(out.astype(np.float32)), res


def kernel(**inputs) -> np.ndarray:
    out, _ = run(inputs, trace=False)
    return out


# revision 29
# speedup vs baseline: 2.3935x; 1.0728x over previous
"""Trainium2 Bass kernel for nn_NodeInfoPropagate (GNN message passing).

Strategy (8 NeuronCores, node-parallel), v2:
  - Shard the 20000 nodes across 8 cores (2500/core, padded to 2560 = 5 tiles
    of 512).  Weights replicated.  Activations live on-chip transposed
    [feature-on-partition, node-on-free] so matmuls chain without transposes.
  - Gathers: parent rows via dma_gather(transpose=True); neighbor rows via
    dma_gather(transpose=False) whose [lane-on-partition, row-major] output
    feeds 0/1-indicator matmuls on the PE that sum each node's 16 neighbor
    rows (replacing the vector-engine reduce).  All gathers round-robin over
    4 SWDGE queues, which generate descriptors concurrently (~3x).
  - gather commutes with linear maps: p[par] = x[par] @ Wp.T etc., so only
    ONE x table is gathered per layer.  For layer 1 it commutes further
    through W_in: x0[par] = feat[par] @ Win.T + b_in, so layer-1 gathers run
    against the (host-replicated) input feature table starting at t=0 and
    the first AllGather is eliminated.  Layers 2..depth gather a bf16 x
    table AllGathered (RDH) into every core's HBM.
"""

import sys

sys.path.insert(0, "/opt/trn_rl_repo")

import numpy as np
import ml_dtypes

import concourse.bass as bass
import concourse.bacc as bacc
import concourse.tile as tile
import concourse.mybir as mybir
from concourse import bass_utils

N = 20000
K = 16
H = 256
DIN = 256
NCORES = 8
NC_REAL = N // NCORES          # 2500 real nodes per core
NT = 512                       # node tile (matmul free dim / PSUM bank)
T = 5                          # tiles per core
NCP = NT * T                   # 2560 padded nodes per core
NTAB = N + 128                 # table rows (pad keeps alignment comfy)
NQ = (NT // 4) * K             # 2048 neighbor idxs per quarter-tile
AGSPLIT = 3 * NT               # local rows in first AllGather chunk (1536)
AGREST = NC_REAL - AGSPLIT     # local rows in second chunk (964)
CHUNKA = NCORES * AGSPLIT      # table rows in first chunk (12288)

F32 = mybir.dt.float32
F32R = mybir.dt.float32r
BF16 = mybir.dt.bfloat16
I16 = mybir.dt.int16
BF = ml_dtypes.bfloat16

_CACHE = {}


def _build(depth: int):
    nc = bacc.Bacc("TRN2", target_bir_lowering=False, debug=False,
                   num_devices=NCORES, num_swdge_queues=4)

    feattab = nc.dram_tensor("feattab", [NTAB, DIN], BF16, kind="ExternalInput")
    featT = nc.dram_tensor("featT", [128, 2, NCP], F32, kind="ExternalInput")
    invcnt = nc.dram_tensor("invcnt", [128, NCP], F32, kind="ExternalInput")
    nbr_idx = nc.dram_tensor("nbr_idx", [128, T, NT], I16, kind="ExternalInput")
    par_idx = nc.dram_tensor("par_idx", [128, T, NT // 16], I16, kind="ExternalInput")
    ind_in = nc.dram_tensor("ind_in", [128, T, NT], BF16, kind="ExternalInput")
    w_in = nc.dram_tensor("w_in", [128, 2, H], F32, kind="ExternalInput")
    w_in_b = nc.dram_tensor("w_in_b", [128, 2, H], BF16, kind="ExternalInput")
    w_ih = nc.dram_tensor("w_ih", [128, 2, 3 * H], BF16, kind="ExternalInput")
    w_hh = nc.dram_tensor("w_hh", [128, 2, 3 * H], F32, kind="ExternalInput")
    w_p = nc.dram_tensor("w_p", [128, 2, H], BF16, kind="ExternalInput")
    w_n = nc.dram_tensor("w_n", [128, 2, H], BF16, kind="ExternalInput")
    # bias columns: 0-1 b_in, 2-3 b_p+b_n, 4-5 b_r, 6-7 b_z, 8-9 b_ih_n,
    # 10-11 b_hh_n  (per 128-feature chunk)
    biases = nc.dram_tensor("biases", [128, 12], F32, kind="ExternalInput")
    ident_b = nc.dram_tensor("ident_b", [128, 128], BF16, kind="ExternalInput")
    y = nc.dram_tensor("y", [NCP, H], F32, kind="ExternalOutput")

    SIG = mybir.ActivationFunctionType.Sigmoid
    TANH = mybir.ActivationFunctionType.Tanh
    ADD = mybir.AluOpType.add
    MULT = mybir.AluOpType.mult

    qctr = [0]

    def qn():
        q = qctr[0] % 4
        qctr[0] += 1
        return q

    with tile.TileContext(nc) as tc:
        with (
            tc.tile_pool(name="const", bufs=1) as constp,
            tc.tile_pool(name="state", bufs=1) as statep,
            tc.tile_pool(name="dram", bufs=1, space="DRAM") as dramp,
            tc.tile_pool(name="gpp", bufs=2) as gpp,
            tc.tile_pool(name="gnp", bufs=8) as gnp,
            tc.tile_pool(name="work", bufs=2) as workp,
            tc.tile_pool(name="tmp", bufs=2) as tmpp,
            tc.tile_pool(name="ps", bufs=2, space="PSUM") as psp,
            tc.tile_pool(name="psn", bufs=1, space="PSUM") as psnp,
            tc.tile_pool(name="psg", bufs=4, space="PSUM") as psgp,
        ):
            # ---- resident constants (gather metadata first) -------------
            nbr_sb = constp.tile([128, T, NT], I16, name="nbr_sb")
            nc.sync.dma_start(nbr_sb[:], nbr_idx.ap())
            par_sb = constp.tile([128, T, NT // 16], I16, name="par_sb")
            nc.sync.dma_start(par_sb[:], par_idx.ap())
            ind_sb = constp.tile([128, T, NT], BF16, name="ind_sb")
            nc.sync.dma_start(ind_sb[:], ind_in.ap())
            win_sb = constp.tile([128, 2, H], F32R, name="win_sb")
            nc.sync.dma_start(win_sb[:], w_in.ap().bitcast(F32R))
            winb_sb = constp.tile([128, 2, H], BF16, name="winb_sb")
            nc.sync.dma_start(winb_sb[:], w_in_b.ap())
            wih_sb = constp.tile([128, 2, 3 * H], BF16, name="wih_sb")
            nc.sync.dma_start(wih_sb[:], w_ih.ap())
            whh_sb = constp.tile([128, 2, 3 * H], F32R, name="whh_sb")
            nc.sync.dma_start(whh_sb[:], w_hh.ap().bitcast(F32R))
            wp_sb = constp.tile([128, 2, H], BF16, name="wp_sb")
            nc.sync.dma_start(wp_sb[:], w_p.ap())
            wn_sb = constp.tile([128, 2, H], BF16, name="wn_sb")
            nc.sync.dma_start(wn_sb[:], w_n.ap())
            bias_sb = constp.tile([128, 12], F32, name="bias_sb")
            nc.sync.dma_start(bias_sb[:], biases.ap())
            idb_sb = constp.tile([128, 128], BF16, name="idb_sb")
            nc.sync.dma_start(idb_sb[:], ident_b.ap())
            feat_sb = constp.tile([128, 2, NCP], F32R, name="feat_sb")
            nc.sync.dma_start(feat_sb[:], featT.ap().bitcast(F32R))
            inv_sb = constp.tile([128, NCP], F32, name="inv_sb")
            nc.sync.dma_start(inv_sb[:], invcnt.ap())

            xF = [statep.tile([128, 2, NCP], BF16, name=f"xF{i}") for i in range(2)]

            xloc = dramp.tile([NCP, H], BF16, name="xloc")
            xtabs = [dramp.tile([NTAB, H], BF16, name=f"xtab{i}")
                     for i in range(2)]

            def mm_f32r(out_ps, lhsT, rhs, start, stop):
                nc.tensor.matmul(out_ps, lhsT, rhs, start=start, stop=stop)

            def write_table_tile(xf, t):
                """transpose tile t of xf (bf16) to row-major, DMA to xloc."""
                for b in range(NT // 128):
                    c0 = t * NT + b * 128
                    rm = workp.tile([128, 2, 128], BF16, tag="rm", name="rm")
                    for c in range(2):
                        pst = psp.tile([128, 128], BF16, tag="sum", name="pst")
                        nc.tensor.transpose(pst[:], xf[:, c, c0:c0 + 128],
                                            idb_sb[:])
                        nc.vector.tensor_copy(rm[:, c, :], pst[:])
                    nc.sync.dma_start(xloc[c0:c0 + 128, :], rm[:])

            def write_output_tile(xf, t):
                ts0 = t * NT
                for b in range(NT // 128):
                    rmf = workp.tile([128, 2, 128], F32, tag="rmf", name="rmf")
                    for c in range(2):
                        pst = psp.tile([128, 128], BF16, tag="sum", name="pstf")
                        nc.tensor.transpose(
                            pst[:], xf[:, c, ts0 + b * 128:ts0 + (b + 1) * 128],
                            idb_sb[:])
                        nc.vector.tensor_copy(rmf[:, c, :], pst[:])
                    r0 = ts0 + b * 128
                    nc.sync.dma_start(y[r0:r0 + 128, :], rmf[:])

            def allgather(xtab, lo, hi, olo, ohi):
                """xloc[lo:hi] from each core -> contiguous xtab[olo:ohi]
                (table rows are chunk-major; host remaps gather indices)."""
                nc.gpsimd.collective_compute(
                    "AllGather", mybir.AluOpType.bypass,
                    replica_groups=[list(range(NCORES))],
                    ins=[xloc[lo:hi, :].opt()],
                    outs=[xtab[olo:ohi, :].opt()],
                )

            def gather_tile(tab, t):
                """parent rows (transposed) + neighbor rows (row-major)."""
                pg = gpp.tile([128, 2, NT], BF16, tag="pgat", name="pg")
                nc.gpsimd.dma_gather(pg[:], tab[:], par_sb[:, t, :],
                                     NT, NT, H, transpose=True,
                                     single_packet=False, queue_num=qn())
                ngs = []
                for qt in range(4):
                    ng = gnp.tile([128, NQ // 128, H], BF16, tag="ng",
                                  name="ng")
                    nc.gpsimd.dma_gather(
                        ng[:], tab[:],
                        nbr_sb[:, t, qt * (NT // 4):(qt + 1) * (NT // 4)],
                        NQ, NQ, H, transpose=False,
                        single_packet=False, queue_num=qn())
                    ngs.append(ng)
                return pg, ngs

            def nbr_sum(t, ngs):
                """0/1-indicator matmuls: sum each node's K neighbor rows.
                Returns two [128, NT] PSUM tiles (feature chunks)."""
                psn = [psnp.tile([128, NT], F32, tag=f"nb{h}", name=f"psn{h}")
                       for h in range(2)]
                for qt, ng in enumerate(ngs):
                    for c in range(NQ // 128):
                        col = qt * (NT // 4) + c * 8
                        for h in range(2):
                            nc.tensor.matmul(
                                psn[h][:, col:col + 8],
                                ng[:, c, h * 128:(h + 1) * 128],
                                ind_sb[:, t, col:col + 8],
                                start=True, stop=True)
                return psn

            def summary_tile(t, par_rhs, nbr_rhs):
                """sT = par_rhs @ Wp.T + nbr_rhs @ Wn.T + (b_p + b_n)."""
                ts = slice(t * NT, (t + 1) * NT)
                sT = workp.tile([128, 2, NT], F32R, tag="sT", name="sT")
                for oc in range(2):
                    ps = psp.tile([128, NT], F32, tag="sum", name="psS")
                    for hc in range(2):
                        nc.tensor.matmul(ps[:],
                                         wp_sb[:, hc, oc * 128:(oc + 1) * 128],
                                         par_rhs[:, hc, :],
                                         start=(hc == 0), stop=False)
                    for hc in range(2):
                        nc.tensor.matmul(ps[:],
                                         wn_sb[:, hc, oc * 128:(oc + 1) * 128],
                                         nbr_rhs[:, hc, :],
                                         start=False, stop=(hc == 1))
                    nc.vector.tensor_scalar_add(sT[:, oc, :], ps[:],
                                                bias_sb[:, 2 + oc:3 + oc])
                return sT

            def gru_tile(t, xf_in, xf_out, sT):
                ts = slice(t * NT, (t + 1) * NT)
                for oc in range(2):
                    rp = psgp.tile([128, NT], F32, tag="gate", name="rp")
                    zp = psgp.tile([128, NT], F32, tag="gate", name="zp")
                    ip = psgp.tile([128, NT], F32, tag="gate", name="ip")
                    hp = psgp.tile([128, NT], F32, tag="gate", name="hp")
                    for gate, pst in ((0, rp), (1, zp)):
                        o0 = gate * H + oc * 128
                        for hc in range(2):
                            mm_f32r(pst[:], wih_sb[:, hc, o0:o0 + 128],
                                    xf_in[:, hc, ts],
                                    start=(hc == 0), stop=False)
                        for hc in range(2):
                            mm_f32r(pst[:], whh_sb[:, hc, o0:o0 + 128],
                                    sT[:, hc, :],
                                    start=False, stop=(hc == 1))
                    o0 = 2 * H + oc * 128
                    for hc in range(2):
                        mm_f32r(ip[:], wih_sb[:, hc, o0:o0 + 128],
                                xf_in[:, hc, ts],
                                start=(hc == 0), stop=(hc == 1))
                    for hc in range(2):
                        mm_f32r(hp[:], whh_sb[:, hc, o0:o0 + 128],
                                sT[:, hc, :],
                                start=(hc == 0), stop=(hc == 1))
                    r = tmpp.tile([128, NT], F32, tag="r", name="r")
                    nc.scalar.activation(r[:], rp[:], SIG,
                                         bias=bias_sb[:, 4 + oc:5 + oc])
                    z = tmpp.tile([128, NT], F32, tag="z", name="z")
                    nc.scalar.activation(z[:], zp[:], SIG,
                                         bias=bias_sb[:, 6 + oc:7 + oc])
                    # n = tanh((i_n + b_ih_n) + r * (h_n + b_hh_n))
                    hnr = tmpp.tile([128, NT], F32, tag="hnr", name="hnr")
                    nc.vector.scalar_tensor_tensor(
                        hnr[:], hp[:], bias_sb[:, 10 + oc:11 + oc], r[:],
                        op0=ADD, op1=MULT)
                    npre = tmpp.tile([128, NT], F32, tag="npre", name="npre")
                    nc.vector.scalar_tensor_tensor(
                        npre[:], ip[:], bias_sb[:, 8 + oc:9 + oc], hnr[:],
                        op0=ADD, op1=ADD)
                    nt_ = tmpp.tile([128, NT], F32, tag="nt", name="nt")
                    nc.scalar.activation(nt_[:], npre[:], TANH)
                    # x_new = n + z * (summary - n)
                    d = tmpp.tile([128, NT], F32, tag="d", name="d")
                    nc.vector.tensor_sub(d[:], sT[:, oc, :].bitcast(F32), nt_[:])
                    dz = tmpp.tile([128, NT], F32, tag="dz", name="dz")
                    nc.vector.tensor_mul(dz[:], d[:], z[:])
                    nc.vector.tensor_add(xf_out[:, oc, ts], dz[:], nt_[:])

            # ---- layer 0: x0 = W_in @ feat + b_in  (local, f32) ---------
            for t in range(T):
                ts = slice(t * NT, (t + 1) * NT)
                for oc in range(2):
                    ps = psp.tile([128, NT], F32, tag="sum", name="ps0")
                    for dc in range(2):
                        mm_f32r(ps[:], win_sb[:, dc, oc * 128:(oc + 1) * 128],
                                feat_sb[:, dc, ts], start=(dc == 0), stop=(dc == 1))
                    nc.vector.tensor_scalar_add(xF[0][:, oc, ts], ps[:],
                                                bias_sb[:, oc:oc + 1])
                if depth == 0:
                    write_output_tile(xF[0], t)

            # ---- GRU layers ---------------------------------------------
            cur = 0
            for layer in range(depth):
                first = layer == 0
                last = layer == depth - 1
                xf_in, xf_out = xF[cur], xF[1 - cur]
                for t in range(T):
                    ts = slice(t * NT, (t + 1) * NT)
                    if first:
                        # gather FEAT rows; x0[g] = feat[g] @ Win.T + b_in
                        pg, ngs = gather_tile(feattab, t)
                        psn = nbr_sum(t, ngs)
                        fng = workp.tile([128, 2, NT], BF16, tag="fng",
                                         name="fng")
                        for h in range(2):
                            nc.vector.tensor_mul(fng[:, h, :], psn[h][:],
                                                 inv_sb[:, ts])
                        x0n = workp.tile([128, 2, NT], BF16, tag="x0n",
                                         name="x0n")
                        x0p = workp.tile([128, 2, NT], BF16, tag="x0p",
                                         name="x0p")
                        for dst, rhs in ((x0n, fng), (x0p, pg)):
                            for oc in range(2):
                                ps = psp.tile([128, NT], F32, tag="sum",
                                              name="psW")
                                for dc in range(2):
                                    nc.tensor.matmul(
                                        ps[:],
                                        winb_sb[:, dc, oc * 128:(oc + 1) * 128],
                                        rhs[:, dc, :],
                                        start=(dc == 0), stop=(dc == 1))
                                nc.vector.tensor_scalar_add(
                                    dst[:, oc, :], ps[:], bias_sb[:, oc:oc + 1])
                        sT = summary_tile(t, x0p, x0n)
                    else:
                        pg, ngs = gather_tile(xtabs[(layer - 1) % 2], t)
                        psn = nbr_sum(t, ngs)
                        nmean = workp.tile([128, 2, NT], BF16, tag="fng",
                                           name="nmean")
                        for h in range(2):
                            nc.vector.tensor_mul(nmean[:, h, :], psn[h][:],
                                                 inv_sb[:, ts])
                        sT = summary_tile(t, pg, nmean)
                    gru_tile(t, xf_in, xf_out, sT)
                    if last:
                        write_output_tile(xf_out, t)
                    else:
                        write_table_tile(xf_out, t)
                        if t == 2:
                            allgather(xtabs[layer % 2], 0, AGSPLIT, 0, CHUNKA)
                if not last:
                    allgather(xtabs[layer % 2], AGSPLIT, NC_REAL, CHUNKA, N)
                cur = 1 - cur

    nc.compile()
    return nc


def _get_nc(depth: int):
    if depth not in _CACHE:
        _CACHE[depth] = _build(depth)
    return _CACHE[depth]


def _idx_layout(lin):
    """linear int16 idx list (len % 16 == 0) -> [128, len//16] wrapped in 16
    partitions, replicated across the 8 gpsimd core groups."""
    v = lin.reshape(-1, 16).T.astype(np.int16)        # [16, len//16]
    return np.tile(v, (8, 1))                         # [128, len//16]


def _chunk2(w):
    """[256, M] -> [128, 2, M] with [p, c, m] = w[c*128+p, m]."""
    M = w.shape[1]
    return np.ascontiguousarray(w.reshape(2, 128, M).transpose(1, 0, 2))


def prepare_inputs(inputs):
    """host-side preprocessing: returns in_maps for the 8 cores."""
    adj = np.asarray(inputs["nodeAdjacencySpecTensor"]).astype(np.int64)
    names = np.asarray(inputs["nodeNamesEncoded"], dtype=np.float32)
    attrs = np.asarray(inputs["nodeAttributesEncoded"], dtype=np.float32)

    parent = adj[:, 0]
    parent = np.clip(np.where(parent < 0, parent + N, parent), 0, N - 1)
    nbr = adj[:, 1:]
    mask = nbr >= 0
    cnt = np.maximum(mask.sum(1), 1).astype(np.float32)
    safe = np.where(mask, np.clip(nbr, 0, N - 1), 0).astype(np.int64)
    inv = (1.0 / cnt).astype(np.float32)
    # (indices remapped to chunk-major table rows below)

    feat = np.concatenate([names, attrs], axis=1)      # [N, 256] f32

    # chunk-major table row order: global node g = c*2500 + r maps to
    # c*1536 + r (r < 1536) else CHUNKA + c*964 + (r - 1536), so each
    # AllGather chunk's output is a contiguous row range.
    g = np.arange(N)
    gc, gr = g // NC_REAL, g % NC_REAL
    tabrow = np.where(gr < AGSPLIT, gc * AGSPLIT + gr,
                      CHUNKA + gc * AGREST + (gr - AGSPLIT)).astype(np.int64)

    feattab = np.zeros((NTAB, DIN), dtype=BF)
    feattab[tabrow] = feat.astype(BF)
    parent = tabrow[parent]
    safe = tabrow[safe]

    W_in = np.asarray(inputs["W_in"], np.float32)
    W_p = np.asarray(inputs["W_parent"], np.float32)
    W_n = np.asarray(inputs["W_neighbor"], np.float32)
    W_ih = np.asarray(inputs["W_ih"], np.float32)
    W_hh = np.asarray(inputs["W_hh"], np.float32)
    b_in = np.asarray(inputs["b_in"], np.float32)
    b_p = np.asarray(inputs["b_parent"], np.float32)
    b_n = np.asarray(inputs["b_neighbor"], np.float32)
    b_ih = np.asarray(inputs["b_ih"], np.float32)
    b_hh = np.asarray(inputs["b_hh"], np.float32)

    w_in_a = _chunk2(W_in.T)                            # [128, 2, 256]
    w_ih_a = _chunk2(W_ih.T).astype(BF)                 # [128, 2, 768]
    w_hh_a = _chunk2(W_hh.T)
    w_p_a = _chunk2(W_p.T).astype(BF)
    w_n_a = _chunk2(W_n.T).astype(BF)

    bias = np.zeros((128, 12), np.float32)
    for col, vec in ((0, b_in), (2, b_p + b_n), (4, (b_ih + b_hh)[0:H]),
                     (6, (b_ih + b_hh)[H:2 * H]), (8, b_ih[2 * H:3 * H]),
                     (10, b_hh[2 * H:3 * H])):
        bias[:, col] = vec[0:128]
        bias[:, col + 1] = vec[128:256]

    ident_b = np.eye(128, dtype=BF)

    shared = dict(feattab=feattab, w_in=w_in_a,
                  w_in_b=w_in_a.astype(BF), w_ih=w_ih_a, w_hh=w_hh_a,
                  w_p=w_p_a, w_n=w_n_a, biases=bias, ident_b=ident_b)

    # indicator row for (node-in-tile n, slot k): partition (n%8)*16 + k
    ind_rows = (np.arange(NT)[:, None] % 8) * 16 + np.arange(K)[None, :]

    in_maps = []
    for c in range(NCORES):
        g0 = c * NC_REAL
        # features, transposed + padded
        f = np.zeros((NCP, DIN), np.float32)
        f[:NC_REAL] = feat[g0:g0 + NC_REAL]
        featT_c = np.ascontiguousarray(
            f.T.reshape(2, 128, NCP).transpose(1, 0, 2))
        # inv count broadcast
        iv = np.ones(NCP, np.float32)
        iv[:NC_REAL] = inv[g0:g0 + NC_REAL]
        inv_c = np.broadcast_to(iv, (128, NCP)).copy()
        # indices (all clipped to valid rows; masking via indicator)
        par = np.zeros(NCP, np.int64)
        par[:NC_REAL] = parent[g0:g0 + NC_REAL]
        nbrs = np.zeros((NCP, K), np.int64)
        nbrs[:NC_REAL] = safe[g0:g0 + NC_REAL]
        msk = np.zeros((NCP, K), np.float32)
        msk[:NC_REAL] = mask[g0:g0 + NC_REAL]
        nbr_t = np.zeros((128, T, NT), np.int16)
        par_t = np.zeros((128, T, NT // 16), np.int16)
        ind_t = np.zeros((128, T, NT), dtype=BF)
        for t in range(T):
            nbr_t[:, t, :] = _idx_layout(nbrs[t * NT:(t + 1) * NT].reshape(-1))
            par_t[:, t, :] = _idx_layout(par[t * NT:(t + 1) * NT])
            m = msk[t * NT:(t + 1) * NT]               # [NT, K]
            M = np.zeros((128, NT), np.float32)
            M[ind_rows.ravel(), np.repeat(np.arange(NT), K)] = m.ravel()
            ind_t[:, t, :] = M.astype(BF)
        in_maps.append(dict(featT=featT_c, invcnt=inv_c, nbr_idx=nbr_t,
                            par_idx=par_t, ind_in=ind_t, **shared))
    return in_maps


def run(inputs, trace=False, **kw):
    depth = int(np.asarray(inputs["depth"]))
    nc = _get_nc(depth)
    in_maps = prepare_inputs(inputs)
    res = bass_utils.run_bass_kernel_spmd(nc, in_maps,
                                          core_ids=list(range(NCORES)),
                                          trace=trace, **kw)
    out = np.concatenate([np.asarray(res.results[c]["y"])[:NC_REAL]
                          for c in range(NCORES)], axis=0)
    return np.ascontiguousarray(out.astype(np.float32)), res


def kernel(**inputs) -> np.ndarray:
    out, _ = run(inputs, trace=False)
    return out
